# revision 15
# baseline (speedup 1.0000x reference)
"""Cross-attention (B=4, Lq=Lkv=4096, D=512, single head) on 8 Trainium2 NeuronCores.

Sharding: data-parallel over batch (4) x sequence-parallel over Lq (2) = 8 shards.
K/V work is replicated within each batch pair (cheap: ~20% of FLOPs).

Per-core kernel (Bass/Tile), everything kept "transposed" so no on-chip
transposes are ever needed:
  QT[e, q]  = WqT.T-contraction:  QT = sum_d WqT[d,e] * xT[d,q]      (+bq)
  KT[e, k]  = likewise from yT                                        (+bk)
  V [k, e]  = sum_d yT[d,k] * WvT[d,e]                                (+bv)
  ST[k, q]  = sum_e KT[e,k] * QT[e,q]            (= scores^T)
  PT[k, q]  = exp(ST * 1/sqrt(D))                (softmax numerator; no max-sub:
                                                  scores ~ N(0,1) by construction)
  ctxT[e,q] = sum_k V[k,e] * PT[k,q]             (PV matmul, V stationary)
  rs[q]     = sum_k PT[k,q]                      (ones-vector matmul)
  out[q,eo] = (sum_e ctxT[e,q]*WoT[e,eo]) / rs[q] + bo[eo]
"""

import math
import os
import sys

import numpy as np

for _p in ("/opt/trn_rl_repo", "/root/.axon_site/_ro/trn_rl_repo"):
    if os.path.isdir(_p) and _p not in sys.path:
        sys.path.append(_p)

import ml_dtypes  # noqa: E402

import concourse.bacc as bacc  # noqa: E402
import concourse.bass as bass  # noqa: E402
import concourse.tile as tile  # noqa: E402
from concourse import mybir  # noqa: E402
from concourse.bass_utils import run_bass_kernel_spmd  # noqa: E402

P = 128
F32 = mybir.dt.float32
F32R = mybir.dt.float32r
BF16 = mybir.dt.bfloat16

B, LQ, LKV, D = 4, 4096, 4096, 512
N_CORES = 8
LQS = B * LQ // N_CORES  # 2048 q rows per core
QCH = 512  # q chunk (psum free dim)

# matmul dtype mode:
#   bf16   - all matmuls bf16 (fastest, FWL weight loads)
#   hybrid - QK^T scores in fp32r, PV / out-proj in bf16
#   f32r   - all matmuls fp32r (tf32-like, full speed at N>=256)
#   f32    - exact fp32 (4x slower matmuls; debugging reference)
MODE = os.environ.get("ATTN_MM_MODE", "bf16")

LAST_EXEC_NS = None
LAST_RESULTS = None


def _mode_dtypes(mode):
    # (dram/proj dtype, score sbuf dtype, pv sbuf dtype, cast-to-f32r flags)
    if mode == "bf16":
        return dict(proj=BF16, score=BF16, pv=BF16, proj_r=False, score_r=False,
                    pv_r=False)
    if mode == "hybrid":
        return dict(proj=F32, score=F32, pv=BF16, proj_r=True, score_r=True,
                    pv_r=False)
    if mode == "f32r":
        return dict(proj=F32, score=F32, pv=F32, proj_r=True, score_r=True,
                    pv_r=True)
    if mode == "f32":
        return dict(proj=F32, score=F32, pv=F32, proj_r=False, score_r=False,
                    pv_r=False)
    raise ValueError(mode)


def _r(ap, flag):
    """Bitcast an fp32 access pattern to fp32r (reduced-precision matmul)."""
    return ap.bitcast(F32R) if flag else ap


def build_program(Lq=LQS, Lkv=LKV, Dd=D, mode=MODE):
    """Build the single-core SPMD program. Returns (nc, meta)."""
    dt = _mode_dtypes(mode)
    DTL = Dd // P           # number of 128-partition tiles along D
    NQC = Lq // QCH         # q chunks
    NKT = Lkv // P          # 128-row k tiles
    NKC = Lkv // QCH        # 512-col k chunks (for KT/V projection)
    NQS = QCH // P          # q subtiles per chunk (=4)
    scale = 1.0 / math.sqrt(Dd)

    nc = bacc.Bacc("TRN2", target_bir_lowering=False, debug=False)

    xT = nc.dram_tensor("xT", [Dd, Lq], dt["proj"], kind="ExternalInput")
    yT = nc.dram_tensor("yT", [Dd, Lkv], dt["proj"], kind="ExternalInput")
    wqT = nc.dram_tensor("wqT", [Dd, Dd], dt["proj"], kind="ExternalInput")
    wkT = nc.dram_tensor("wkT", [Dd, Dd], dt["proj"], kind="ExternalInput")
    wvT = nc.dram_tensor("wvT", [Dd, Dd], dt["proj"], kind="ExternalInput")
    woT = nc.dram_tensor("woT", [Dd, Dd], dt["pv"], kind="ExternalInput")
    bq = nc.dram_tensor("bq", [Dd], F32, kind="ExternalInput")
    bk = nc.dram_tensor("bk", [Dd], F32, kind="ExternalInput")
    bv = nc.dram_tensor("bv", [Dd], F32, kind="ExternalInput")
    bo = nc.dram_tensor("bo", [Dd], F32, kind="ExternalInput")
    out = nc.dram_tensor("out", [Lq, Dd], F32, kind="ExternalOutput")

    xT_r = xT[:].rearrange("(do di) q -> di do q", di=P)
    yT_r = yT[:].rearrange("(do di) k -> di do k", di=P)

    def bcast(ap_1d, parts=P):
        # broadcast a [D] dram vector across partitions -> [parts, D] AP
        return bass.AP(tensor=ap_1d.tensor, offset=ap_1d.offset,
                       ap=[[0, parts], list(ap_1d.ap[0])])

    ID = mybir.ActivationFunctionType.Identity
    EXP = mybir.ActivationFunctionType.Exp
    ADD = mybir.AluOpType.add
    MULT = mybir.AluOpType.mult

    with tile.TileContext(nc) as tc:
        with (
            tc.tile_pool(name="consts", bufs=1) as consts,
            tc.tile_pool(name="bigs", bufs=1) as bigs,
        ):
            # ---- constants (emission order = DMA priority: QT path first) ----
            bq_sb = consts.tile([P, DTL], F32)
            bk_sb = consts.tile([P, DTL], F32)
            ones_f32 = consts.tile([P, 1], F32)
            nc.vector.memset(ones_f32, 1.0)
            one_f32 = consts.tile([1, 1], F32)
            nc.vector.memset(one_f32, 1.0)
            wo_sb = consts.tile([P, DTL, Dd], dt["pv"])
            bv_rep = consts.tile([P, Dd], F32)
            bo_rep = consts.tile([P, Dd], F32)

            # ---- persistent activations ----
            QT_sb = bigs.tile([P, DTL, Lq], dt["score"])
            KT_sb = bigs.tile([P, DTL, Lkv], dt["score"])
            V_sb = bigs.tile([P, NKT, Dd], dt["pv"])

            # ================= projections =================
            with (
                tc.tile_pool(name="pw", bufs=1) as pw,
                tc.tile_pool(name="pin", bufs=2) as pin,
                tc.tile_pool(name="pps", bufs=3, space="PSUM") as pps,
            ):
                wq_sb = pw.tile([P, DTL, Dd], dt["proj"])
                wk_sb = pw.tile([P, DTL, Dd], dt["proj"])
                wv_sb = pw.tile([P, DTL, Dd], dt["proj"])
                nc.sync.dma_start(wq_sb,
                                  wqT[:].rearrange("(do di) e -> di do e", di=P))
                with nc.allow_non_contiguous_dma(reason="tiny bias reshape"):
                    nc.gpsimd.dma_start(bq_sb,
                                        bq[:].rearrange("(eo ei) -> ei eo", ei=P))
                    nc.gpsimd.dma_start(bk_sb,
                                        bk[:].rearrange("(eo ei) -> ei eo", ei=P))

                # QT[e, q] (+bq)
                for qc in range(NQC):
                    xt = pin.tile([P, DTL, QCH], dt["proj"], tag="xt")
                    nc.sync.dma_start(xt, xT_r[:, :, qc * QCH:(qc + 1) * QCH])
                    for es in range(DTL):
                        ps = pps.tile([P, QCH], F32, tag="ps")
                        for dti in range(DTL):
                            nc.tensor.matmul(
                                ps,
                                _r(wq_sb[:, dti, es * P:(es + 1) * P], dt["proj_r"]),
                                _r(xt[:, dti, :], dt["proj_r"]),
                                start=(dti == 0), stop=(dti == DTL - 1))
                        nc.scalar.activation(
                            QT_sb[:, es, qc * QCH:(qc + 1) * QCH], ps, ID,
                            bias=bq_sb[:, es:es + 1])

                nc.sync.dma_start(wk_sb,
                                  wkT[:].rearrange("(do di) e -> di do e", di=P))
                nc.sync.dma_start(wv_sb,
                                  wvT[:].rearrange("(do di) e -> di do e", di=P))
                nc.gpsimd.dma_start(bv_rep, bcast(bv[:]))
                nc.sync.dma_start(wo_sb,
                                  woT[:].rearrange("(eo ei) f -> ei eo f", ei=P))
                nc.gpsimd.dma_start(bo_rep, bcast(bo[:]))

                # KT[e, k] (+bk) and V[k, e] (+bv), per 512-wide k chunk
                for kc in range(NKC):
                    yt = pin.tile([P, DTL, QCH], dt["proj"], tag="yt")
                    nc.sync.dma_start(yt, yT_r[:, :, kc * QCH:(kc + 1) * QCH])
                    for es in range(DTL):
                        ps = pps.tile([P, QCH], F32, tag="ps")
                        for dti in range(DTL):
                            nc.tensor.matmul(
                                ps,
                                _r(wk_sb[:, dti, es * P:(es + 1) * P], dt["proj_r"]),
                                _r(yt[:, dti, :], dt["proj_r"]),
                                start=(dti == 0), stop=(dti == DTL - 1))
                        nc.scalar.activation(
                            KT_sb[:, es, kc * QCH:(kc + 1) * QCH], ps, ID,
                            bias=bk_sb[:, es:es + 1])
                    for ki in range(QCH // P):
                        ks = kc * (QCH // P) + ki
                        ps = pps.tile([P, Dd], F32, tag="ps")
                        for dti in range(DTL):
                            nc.tensor.matmul(
                                ps,
                                _r(yt[:, dti, ki * P:(ki + 1) * P], dt["proj_r"]),
                                _r(wv_sb[:, dti, :], dt["proj_r"]),
                                start=(dti == 0), stop=(dti == DTL - 1))
                        nc.vector.tensor_add(V_sb[:, ks, :], ps, bv_rep)

            # ================= attention =================
            with (
                tc.tile_pool(name="ptp", bufs=6) as ptp,
                tc.tile_pool(name="attsb", bufs=2) as attsb,
                tc.tile_pool(name="outsb", bufs=3) as outsb,
                tc.tile_pool(name="stp", bufs=2, space="PSUM") as stp,
                tc.tile_pool(name="ctxp", bufs=1, space="PSUM") as ctxp,
                tc.tile_pool(name="rsp", bufs=1, space="PSUM") as rsp,
            ):
                outp = stp  # "ops" shares the 2 "st" psum slots via its own tag
                for qc in range(NQC):
                    q_sl = slice(qc * QCH, (qc + 1) * QCH)
                    ctx_ps = ctxp.tile([P, DTL, QCH], F32, tag="ctx")
                    rs_ps = rsp.tile([1, QCH], F32, tag="rs")
                    ptsum = attsb.tile([P, QCH], F32, tag="ptsum")

                    prev = None
                    for k in range(NKT + 1):
                        cur = None
                        if k < NKT:
                            st = stp.tile([P, QCH], F32, tag="st")
                            for es in range(DTL):
                                nc.tensor.matmul(
                                    st,
                                    _r(KT_sb[:, es, k * P:(k + 1) * P],
                                       dt["score_r"]),
                                    _r(QT_sb[:, es, q_sl], dt["score_r"]),
                                    start=(es == 0), stop=(es == DTL - 1))
                            pt = ptp.tile([P, QCH], dt["pv"], tag="pt")
                            nc.scalar.activation(pt, st, EXP, scale=scale)
                            cur = (k, pt)
                        if prev is not None:
                            pk, ptile = prev
                            # rowsum partial on the (otherwise idle) VectorE
                            if pk == 0:
                                nc.vector.tensor_copy(ptsum, ptile)
                            else:
                                nc.vector.tensor_add(ptsum, ptsum, ptile)
                            for es in range(DTL):
                                nc.tensor.matmul(
                                    ctx_ps[:, es, :],
                                    _r(V_sb[:, pk, es * P:(es + 1) * P],
                                       dt["pv_r"]),
                                    _r(ptile, dt["pv_r"]),
                                    start=(pk == 0), stop=(pk == NKT - 1))
                        prev = cur

                    # collapse the 128-partition ptsum with one f32 matmul
                    nc.tensor.matmul(rs_ps, ones_f32, ptsum,
                                     start=True, stop=True)

                    # rowsum -> per-partition reciprocal [128, NQS].
                    # Transpose the [1, 512] rowsum onto partitions with NQS
                    # tiny matmuls: out[:, qs] = rs_row_slice.T @ [[1.0]].
                    # (Cross-partition SBUF DMAs and internal-DRAM bounces both
                    # fail under this runtime; the PE path is reliable.)
                    rs_sb = attsb.tile([1, QCH], F32, tag="rssb")
                    nc.vector.tensor_copy(rs_sb, rs_ps)
                    rsT_ps = rsp.tile([P, NQS], F32, tag="rsT")
                    for qs in range(NQS):
                        nc.tensor.matmul(
                            rsT_ps[:, qs:qs + 1],
                            rs_sb[0:1, qs * P:(qs + 1) * P], one_f32,
                            start=True, stop=True, skip_group_check=True)
                    recip = attsb.tile([P, NQS], F32, tag="recip")
                    nc.vector.reciprocal(recip, rsT_ps)

                    # ctxT to SBUF
                    ctxT = attsb.tile([P, DTL, QCH], dt["pv"], tag="ctxT")
                    for es in range(DTL):
                        nc.vector.tensor_copy(ctxT[:, es, :], ctx_ps[:, es, :])

                    # out projection + normalize + bias. The psum tile reuses
                    # the freed ctx banks (same tag) so the "st" slots stay
                    # free and the next chunk's ST matmuls start immediately.
                    ops_all = ctxp.tile([P, NQS, Dd], F32, tag="ctx")
                    for qs in range(NQS):
                        ops = ops_all[:, qs, :]
                        for es in range(DTL):
                            nc.tensor.matmul(
                                ops,
                                _r(ctxT[:, es, qs * P:(qs + 1) * P], dt["pv_r"]),
                                _r(wo_sb[:, es, :], dt["pv_r"]),
                                start=(es == 0), stop=(es == DTL - 1))
                        ot = outsb.tile([P, Dd], F32, tag="ot")
                        nc.vector.scalar_tensor_tensor(
                            ot, ops, recip[:, qs:qs + 1], bo_rep,
                            op0=MULT, op1=ADD)
                        row0 = (qc * NQS + qs) * P
                        nc.sync.dma_start(out[row0:row0 + P, :], ot)

    nc.compile()
    return nc


def _np_dt(mdt):
    return {F32: np.float32, BF16: ml_dtypes.bfloat16}[mdt]


def _prep(a, mdt):
    return np.ascontiguousarray(a, dtype=_np_dt(mdt))


def make_in_maps(x, y, Wq, bq, Wk, bk, Wv, bv, Wo, bo, mode=MODE):
    dt = _mode_dtypes(mode)
    pj, pv = _np_dt(dt["proj"]), _np_dt(dt["pv"])
    shared = {
        "wqT": _prep(Wq.T, dt["proj"]),
        "wkT": _prep(Wk.T, dt["proj"]),
        "wvT": _prep(Wv.T, dt["proj"]),
        "woT": _prep(Wo.T, dt["pv"]),
        "bq": _prep(bq, F32), "bk": _prep(bk, F32),
        "bv": _prep(bv, F32), "bo": _prep(bo, F32),
    }
    yTs = [np.ascontiguousarray(y[b].T, dtype=pj) for b in range(B)]
    in_maps = []
    for c in range(N_CORES):
        b, h = divmod(c, N_CORES // B)
        xTc = np.ascontiguousarray(x[b, h * LQS:(h + 1) * LQS, :].T, dtype=pj)
        in_maps.append({"xT": xTc, "yT": yTs[b], **shared})
    return in_maps


_PROG_CACHE = {}


def kernel(x, y, Wq, bq, Wk, bk, Wv, bv, Wo, bo, _trace=False):
    global LAST_EXEC_NS, LAST_RESULTS
    x = np.asarray(x, dtype=np.float32)
    y = np.asarray(y, dtype=np.float32)
    args = [np.asarray(a, dtype=np.float32) for a in (Wq, bq, Wk, bk, Wv, bv, Wo, bo)]

    if MODE not in _PROG_CACHE:
        _PROG_CACHE[MODE] = build_program(LQS, LKV, D, MODE)
    nc = _PROG_CACHE[MODE]

    in_maps = make_in_maps(x, y, *args, mode=MODE)
    res = run_bass_kernel_spmd(nc, in_maps, core_ids=list(range(N_CORES)),
                               trace=_trace)
    LAST_EXEC_NS = res.exec_time_ns
    LAST_RESULTS = res

    out = np.empty((B, LQ, D), dtype=np.float32)
    for c in range(N_CORES):
        b, h = divmod(c, N_CORES // B)
        out[b, h * LQS:(h + 1) * LQS, :] = res.results[c]["out"]
    return out


# revision 21
# speedup vs baseline: 1.0302x; 1.0302x over previous
"""Cross-attention (B=4, Lq=Lkv=4096, D=512, single head) on 8 Trainium2 NeuronCores.

Sharding: data-parallel over batch (4) x sequence-parallel over Lq (2) = 8 shards.
K/V work is replicated within each batch pair (cheap: ~20% of FLOPs).

Per-core kernel (Bass/Tile), everything kept "transposed" so no on-chip
transposes are ever needed:
  QT[e, q]  = WqT.T-contraction:  QT = sum_d WqT[d,e] * xT[d,q]      (+bq)
  KT[e, k]  = likewise from yT                                        (+bk)
  V [k, e]  = sum_d yT[d,k] * WvT[d,e]                                (+bv)
  ST[k, q]  = sum_e KT[e,k] * QT[e,q]            (= scores^T)
  PT[k, q]  = exp(ST * 1/sqrt(D))                (softmax numerator; no max-sub:
                                                  scores ~ N(0,1) by construction)
  ctxT[e,q] = sum_k V[k,e] * PT[k,q]             (PV matmul, V stationary)
  rs[q]     = sum_k PT[k,q]                      (ones-vector matmul)
  out[q,eo] = (sum_e ctxT[e,q]*WoT[e,eo]) / rs[q] + bo[eo]
"""

import math
import os
import sys

import numpy as np

for _p in ("/opt/trn_rl_repo", "/root/.axon_site/_ro/trn_rl_repo"):
    if os.path.isdir(_p) and _p not in sys.path:
        sys.path.append(_p)

import ml_dtypes  # noqa: E402

import concourse.bacc as bacc  # noqa: E402
import concourse.bass as bass  # noqa: E402
import concourse.tile as tile  # noqa: E402
from concourse import mybir  # noqa: E402
from concourse.bass_utils import run_bass_kernel_spmd  # noqa: E402

P = 128
F32 = mybir.dt.float32
F32R = mybir.dt.float32r
BF16 = mybir.dt.bfloat16

B, LQ, LKV, D = 4, 4096, 4096, 512
N_CORES = 8
LQS = B * LQ // N_CORES  # 2048 q rows per core
QCH = 512  # q chunk (psum free dim)

# matmul dtype mode:
#   bf16   - all matmuls bf16 (fastest, FWL weight loads)
#   hybrid - QK^T scores in fp32r, PV / out-proj in bf16
#   f32r   - all matmuls fp32r (tf32-like, full speed at N>=256)
#   f32    - exact fp32 (4x slower matmuls; debugging reference)
MODE = os.environ.get("ATTN_MM_MODE", "bf16")

LAST_EXEC_NS = None
LAST_RESULTS = None


def _mode_dtypes(mode):
    # (dram/proj dtype, score sbuf dtype, pv sbuf dtype, cast-to-f32r flags)
    if mode == "bf16":
        return dict(proj=BF16, score=BF16, pv=BF16, proj_r=False, score_r=False,
                    pv_r=False)
    if mode == "hybrid":
        return dict(proj=F32, score=F32, pv=BF16, proj_r=True, score_r=True,
                    pv_r=False)
    if mode == "f32r":
        return dict(proj=F32, score=F32, pv=F32, proj_r=True, score_r=True,
                    pv_r=True)
    if mode == "f32":
        return dict(proj=F32, score=F32, pv=F32, proj_r=False, score_r=False,
                    pv_r=False)
    raise ValueError(mode)


def _r(ap, flag):
    """Bitcast an fp32 access pattern to fp32r (reduced-precision matmul)."""
    return ap.bitcast(F32R) if flag else ap


def build_program(Lq=LQS, Lkv=LKV, Dd=D, mode=MODE):
    """Build the single-core SPMD program. Returns (nc, meta)."""
    dt = _mode_dtypes(mode)
    DTL = Dd // P           # number of 128-partition tiles along D
    NQC = Lq // QCH         # q chunks
    NKT = Lkv // P          # 128-row k tiles
    NKC = Lkv // QCH        # 512-col k chunks (for KT/V projection)
    NQS = QCH // P          # q subtiles per chunk (=4)
    scale = 1.0 / math.sqrt(Dd)

    nc = bacc.Bacc("TRN2", target_bir_lowering=False, debug=False)

    xT = nc.dram_tensor("xT", [Dd, Lq], dt["proj"], kind="ExternalInput")
    yT = nc.dram_tensor("yT", [Dd, Lkv], dt["proj"], kind="ExternalInput")
    wqT = nc.dram_tensor("wqT", [Dd, Dd], dt["proj"], kind="ExternalInput")
    wkT = nc.dram_tensor("wkT", [Dd, Dd], dt["proj"], kind="ExternalInput")
    wvT = nc.dram_tensor("wvT", [Dd, Dd], dt["proj"], kind="ExternalInput")
    woT = nc.dram_tensor("woT", [Dd, Dd], dt["pv"], kind="ExternalInput")
    bq = nc.dram_tensor("bq", [Dd], F32, kind="ExternalInput")
    bk = nc.dram_tensor("bk", [Dd], F32, kind="ExternalInput")
    bv = nc.dram_tensor("bv", [Dd], F32, kind="ExternalInput")
    bo = nc.dram_tensor("bo", [Dd], F32, kind="ExternalInput")
    out = nc.dram_tensor("out", [Lq, Dd], F32, kind="ExternalOutput")

    xT_r = xT[:].rearrange("(do di) q -> di do q", di=P)
    yT_r = yT[:].rearrange("(do di) k -> di do k", di=P)

    def bcast(ap_1d, parts=P):
        # broadcast a [D] dram vector across partitions -> [parts, D] AP
        return bass.AP(tensor=ap_1d.tensor, offset=ap_1d.offset,
                       ap=[[0, parts], list(ap_1d.ap[0])])

    ID = mybir.ActivationFunctionType.Identity
    EXP = mybir.ActivationFunctionType.Exp
    ADD = mybir.AluOpType.add
    MULT = mybir.AluOpType.mult

    with tile.TileContext(nc) as tc:
        with (
            tc.tile_pool(name="consts", bufs=1) as consts,
            tc.tile_pool(name="bigs", bufs=1) as bigs,
        ):
            # ---- constants (emission order = DMA priority: QT path first) ----
            bq_sb = consts.tile([P, DTL], F32)
            bk_sb = consts.tile([P, DTL], F32)
            ones_f32 = consts.tile([P, 1], F32)
            nc.vector.memset(ones_f32, 1.0)
            one_f32 = consts.tile([1, 1], F32)
            nc.vector.memset(one_f32, 1.0)
            wo_sb = consts.tile([P, DTL, Dd], dt["pv"])
            bv_rep = consts.tile([P, Dd], F32)
            bo_rep = consts.tile([P, Dd], F32)

            # ---- persistent activations ----
            QT_sb = bigs.tile([P, DTL, Lq], dt["score"])
            KT_sb = bigs.tile([P, DTL, Lkv], dt["score"])
            V_sb = bigs.tile([P, NKT, Dd], dt["pv"])

            # ================= projections =================
            with (
                tc.tile_pool(name="pw", bufs=1) as pw,
                tc.tile_pool(name="pin", bufs=2) as pin,
                tc.tile_pool(name="pps", bufs=3, space="PSUM") as pps,
            ):
                wq_sb = pw.tile([P, DTL, Dd], dt["proj"])
                wk_sb = pw.tile([P, DTL, Dd], dt["proj"])
                wv_sb = pw.tile([P, DTL, Dd], dt["proj"])
                wq_r = wqT[:].rearrange("(do di) e -> di do e", di=P)
                for dti in range(DTL):
                    nc.sync.dma_start(wq_sb[:, dti, :], wq_r[:, dti, :])
                with nc.allow_non_contiguous_dma(reason="tiny bias reshape"):
                    nc.gpsimd.dma_start(bq_sb,
                                        bq[:].rearrange("(eo ei) -> ei eo", ei=P))
                    nc.gpsimd.dma_start(bk_sb,
                                        bk[:].rearrange("(eo ei) -> ei eo", ei=P))

                # QT[e, q] (+bq)
                for qc in range(NQC):
                    xt = pin.tile([P, DTL, QCH], dt["proj"], tag="xt")
                    for dti in range(DTL):
                        nc.sync.dma_start(xt[:, dti, :],
                                          xT_r[:, dti, qc * QCH:(qc + 1) * QCH])
                    for es in range(DTL):
                        ps = pps.tile([P, QCH], F32, tag="ps")
                        for dti in range(DTL):
                            nc.tensor.matmul(
                                ps,
                                _r(wq_sb[:, dti, es * P:(es + 1) * P], dt["proj_r"]),
                                _r(xt[:, dti, :], dt["proj_r"]),
                                start=(dti == 0), stop=(dti == DTL - 1))
                        nc.scalar.activation(
                            QT_sb[:, es, qc * QCH:(qc + 1) * QCH], ps, ID,
                            bias=bq_sb[:, es:es + 1])

                nc.sync.dma_start(wk_sb,
                                  wkT[:].rearrange("(do di) e -> di do e", di=P))
                nc.sync.dma_start(wv_sb,
                                  wvT[:].rearrange("(do di) e -> di do e", di=P))
                nc.gpsimd.dma_start(bv_rep, bcast(bv[:]))
                nc.sync.dma_start(wo_sb,
                                  woT[:].rearrange("(eo ei) f -> ei eo f", ei=P))
                nc.gpsimd.dma_start(bo_rep, bcast(bo[:]))

                # KT[e, k] (+bk) and V[k, e] (+bv), per 512-wide k chunk
                for kc in range(NKC):
                    yt = pin.tile([P, DTL, QCH], dt["proj"], tag="yt")
                    nc.sync.dma_start(yt, yT_r[:, :, kc * QCH:(kc + 1) * QCH])
                    for es in range(DTL):
                        ps = pps.tile([P, QCH], F32, tag="ps")
                        for dti in range(DTL):
                            nc.tensor.matmul(
                                ps,
                                _r(wk_sb[:, dti, es * P:(es + 1) * P], dt["proj_r"]),
                                _r(yt[:, dti, :], dt["proj_r"]),
                                start=(dti == 0), stop=(dti == DTL - 1))
                        nc.scalar.activation(
                            KT_sb[:, es, kc * QCH:(kc + 1) * QCH], ps, ID,
                            bias=bk_sb[:, es:es + 1])
                    for ki in range(QCH // P):
                        ks = kc * (QCH // P) + ki
                        ps = pps.tile([P, Dd], F32, tag="ps")
                        for dti in range(DTL):
                            nc.tensor.matmul(
                                ps,
                                _r(yt[:, dti, ki * P:(ki + 1) * P], dt["proj_r"]),
                                _r(wv_sb[:, dti, :], dt["proj_r"]),
                                start=(dti == 0), stop=(dti == DTL - 1))
                        nc.vector.tensor_add(V_sb[:, ks, :], ps, bv_rep)

            # ================= attention =================
            with (
                tc.tile_pool(name="ptp", bufs=6) as ptp,
                tc.tile_pool(name="attsb", bufs=2) as attsb,
                tc.tile_pool(name="outsb", bufs=3) as outsb,
                tc.tile_pool(name="stp", bufs=2, space="PSUM") as stp,
                tc.tile_pool(name="ctxp", bufs=1, space="PSUM") as ctxp,
                tc.tile_pool(name="rsp", bufs=1, space="PSUM") as rsp,
            ):
                outp = stp  # "ops" shares the 2 "st" psum slots via its own tag
                def epilogue(qc, ctx_ps, rs_ps, ptsum):
                    # collapse the 128-partition ptsum with one f32 matmul
                    nc.tensor.matmul(rs_ps, ones_f32, ptsum,
                                     start=True, stop=True)

                    # rowsum -> per-partition reciprocal [128, NQS].
                    # Transpose the [1, 512] rowsum onto partitions with NQS
                    # tiny matmuls: out[:, qs] = rs_row_slice.T @ [[1.0]].
                    # (Cross-partition SBUF DMAs and internal-DRAM bounces both
                    # fail under this runtime; the PE path is reliable.)
                    rs_sb = attsb.tile([1, QCH], F32, tag="rssb")
                    nc.vector.tensor_copy(rs_sb, rs_ps)
                    rsT_ps = rsp.tile([P, NQS], F32, tag="rsT")
                    for qs in range(NQS):
                        nc.tensor.matmul(
                            rsT_ps[:, qs:qs + 1],
                            rs_sb[0:1, qs * P:(qs + 1) * P], one_f32,
                            start=True, stop=True, skip_group_check=True)
                    recip = attsb.tile([P, NQS], F32, tag="recip")
                    nc.vector.reciprocal(recip, rsT_ps)

                    # ctxT to SBUF
                    ctxT = attsb.tile([P, DTL, QCH], dt["pv"], tag="ctxT")
                    for es in range(DTL):
                        nc.vector.tensor_copy(ctxT[:, es, :], ctx_ps[:, es, :])

                    # out projection + normalize + bias
                    for qs in range(NQS):
                        ops = stp.tile([P, Dd], F32, tag="st")
                        for es in range(DTL):
                            nc.tensor.matmul(
                                ops,
                                _r(ctxT[:, es, qs * P:(qs + 1) * P], dt["pv_r"]),
                                _r(wo_sb[:, es, :], dt["pv_r"]),
                                start=(es == 0), stop=(es == DTL - 1))
                        ot = outsb.tile([P, Dd], F32, tag="ot")
                        nc.vector.scalar_tensor_tensor(
                            ot, ops, recip[:, qs:qs + 1], bo_rep,
                            op0=MULT, op1=ADD)
                        row0 = (qc * NQS + qs) * P
                        nc.sync.dma_start(out[row0:row0 + P, :], ot)

                # Chunks are software-pipelined: chunk qc's epilogue (which has
                # serial PE<->DVE round-trips) is emitted after the first ST
                # iterations of chunk qc+1, so the PE stream never drains.
                pending = None
                for qc in range(NQC):
                    q_sl = slice(qc * QCH, (qc + 1) * QCH)
                    ctx_ps = ctxp.tile([P, DTL, QCH], F32, tag="ctx")
                    rs_ps = rsp.tile([1, QCH], F32, tag="rs")
                    ptsum = attsb.tile([P, QCH], F32, tag="ptsum")

                    prev = None
                    for k in range(NKT + 1):
                        cur = None
                        if k < NKT:
                            st = stp.tile([P, QCH], F32, tag="st")
                            for es in range(DTL):
                                nc.tensor.matmul(
                                    st,
                                    _r(KT_sb[:, es, k * P:(k + 1) * P],
                                       dt["score_r"]),
                                    _r(QT_sb[:, es, q_sl], dt["score_r"]),
                                    start=(es == 0), stop=(es == DTL - 1))
                            pt = ptp.tile([P, QCH], dt["pv"], tag="pt")
                            nc.scalar.activation(pt, st, EXP, scale=scale)
                            cur = (k, pt)
                        if k == 1 and pending is not None:
                            # flush the previous chunk's epilogue here: its PE
                            # ops precede this chunk's PV in the FIFO and its
                            # DVE round-trips hide behind ST work.
                            epilogue(*pending)
                            pending = None
                        if prev is not None:
                            pk, ptile = prev
                            # rowsum partial on the (otherwise idle) VectorE
                            if pk == 0:
                                nc.vector.tensor_copy(ptsum, ptile)
                            else:
                                nc.vector.tensor_add(ptsum, ptsum, ptile)
                            for es in range(DTL):
                                nc.tensor.matmul(
                                    ctx_ps[:, es, :],
                                    _r(V_sb[:, pk, es * P:(es + 1) * P],
                                       dt["pv_r"]),
                                    _r(ptile, dt["pv_r"]),
                                    start=(pk == 0), stop=(pk == NKT - 1))
                        prev = cur
                    pending = (qc, ctx_ps, rs_ps, ptsum)
                epilogue(*pending)

    nc.compile()
    return nc


def _np_dt(mdt):
    return {F32: np.float32, BF16: ml_dtypes.bfloat16}[mdt]


def _prep(a, mdt):
    return np.ascontiguousarray(a, dtype=_np_dt(mdt))


def make_in_maps(x, y, Wq, bq, Wk, bk, Wv, bv, Wo, bo, mode=MODE):
    dt = _mode_dtypes(mode)
    pj, pv = _np_dt(dt["proj"]), _np_dt(dt["pv"])
    shared = {
        "wqT": _prep(Wq.T, dt["proj"]),
        "wkT": _prep(Wk.T, dt["proj"]),
        "wvT": _prep(Wv.T, dt["proj"]),
        "woT": _prep(Wo.T, dt["pv"]),
        "bq": _prep(bq, F32), "bk": _prep(bk, F32),
        "bv": _prep(bv, F32), "bo": _prep(bo, F32),
    }
    yTs = [np.ascontiguousarray(y[b].T, dtype=pj) for b in range(B)]
    in_maps = []
    for c in range(N_CORES):
        b, h = divmod(c, N_CORES // B)
        xTc = np.ascontiguousarray(x[b, h * LQS:(h + 1) * LQS, :].T, dtype=pj)
        in_maps.append({"xT": xTc, "yT": yTs[b], **shared})
    return in_maps


_PROG_CACHE = {}


def kernel(x, y, Wq, bq, Wk, bk, Wv, bv, Wo, bo, _trace=False):
    global LAST_EXEC_NS, LAST_RESULTS
    x = np.asarray(x, dtype=np.float32)
    y = np.asarray(y, dtype=np.float32)
    args = [np.asarray(a, dtype=np.float32) for a in (Wq, bq, Wk, bk, Wv, bv, Wo, bo)]

    if MODE not in _PROG_CACHE:
        _PROG_CACHE[MODE] = build_program(LQS, LKV, D, MODE)
    nc = _PROG_CACHE[MODE]

    in_maps = make_in_maps(x, y, *args, mode=MODE)
    res = run_bass_kernel_spmd(nc, in_maps, core_ids=list(range(N_CORES)),
                               trace=_trace)
    LAST_EXEC_NS = res.exec_time_ns
    LAST_RESULTS = res

    out = np.empty((B, LQ, D), dtype=np.float32)
    for c in range(N_CORES):
        b, h = divmod(c, N_CORES // B)
        out[b, h * LQS:(h + 1) * LQS, :] = res.results[c]["out"]
    return out
